# revision 12
# baseline (speedup 1.0000x reference)
"""Causal self-attention (RoPE) Trainium2 Bass kernel, v2.

Sharding: 8 cores = 2 (batch) x 4 (head groups). Each core computes one batch
element b and 4 of the 16 heads end-to-end (QKV projection -> RoPE -> causal
attention -> c_proj rows), producing a partial [T, C] output; the host sums
the 4 partials per batch element (the "all-reduce" of the row-sharded c_proj).

v2 changes vs v1 (baseline 247us):
- All matmul operands in bf16 (x, W, q, k, v, p, y): halves LDWEIGHTS bytes,
  input DMA, and PE power (less clock throttling), with f32 PSUM accumulate.
- q/k stored interleaved per head ([h0 d0:64 | h1 d0:64] per 128-partition
  tile) so score matmuls contract 64 rows per instruction instead of 32:
  halves the score matmul count AND row count (free-dim cost model).
- RoPE writes the interleaved layout directly: 2 full-width muls by cos /
  signed-sin tables plus 4 narrow bf16 adds with the partition-offset-XOR-32
  source (cross-partition-base DVE operands).
- Causal triangle via a bf16 0/1-mask multiply on DVE (replaces gpsimd
  affine_select); the fully-masked lead of diagonal blocks is skipped in the
  exp/AV width instead of memset-to-zero (except the last block, which runs
  full width over a memset lead to close the PSUM accumulation group).
- Softmax normalization: ones-column moved to slot 0 so 1/sumexp reads PSUM
  partition 0 directly; partition-broadcast via the gpsimd custom op instead
  of a DRAM-bounce DMA.
- Output partials in bf16 (host accumulates in f32).
"""

import os
import sys
import numpy as np

N_CORES = 8
B, T, C = 2, 2048, 1024
H = 16
HD = 64
HPC = 4            # heads per core
NT = 4             # token tiles of 512
TQ = 512           # tq tile size
KC = C // 128      # contraction chunks for qkv projection

_PROGRAM_CACHE = {}


def _build_program():
    import concourse.bass as bass
    import concourse.mybir as mybir
    import concourse.bacc as bacc
    import concourse.tile as tile

    F32 = mybir.dt.float32
    BF16 = mybir.dt.bfloat16

    nc = bacc.Bacc("TRN2", target_bir_lowering=False, debug=False,
                   num_devices=N_CORES)

    xT = nc.dram_tensor("xT", [128, NT, KC, TQ], BF16,
                        kind="ExternalInput").ap()
    wqk = nc.dram_tensor("wqk", [128, KC, 512], BF16,
                         kind="ExternalInput").ap()
    wv = nc.dram_tensor("wv", [128, KC, 256], BF16,
                        kind="ExternalInput").ap()
    cos4 = nc.dram_tensor("cos4", [128, T], F32, kind="ExternalInput").ap()
    sing = nc.dram_tensor("sing", [128, T], F32, kind="ExternalInput").ap()
    wp = nc.dram_tensor("wp", [128, 2, C], BF16, kind="ExternalInput").ap()
    trid = nc.dram_tensor("trid", [128, 128], BF16, kind="ExternalInput").ap()
    out = nc.dram_tensor("out", [T, C], BF16, kind="ExternalOutput").ap()

    Exp = mybir.ActivationFunctionType.Exp
    scale = 1.0 / float(np.sqrt(HD))

    with tile.TileContext(nc) as tc:
        with (
            tc.tile_pool(name="const", bufs=1) as const,
            tc.tile_pool(name="xp", bufs=3) as xp,
            tc.tile_pool(name="qk", bufs=1) as qkp,
            tc.tile_pool(name="vaug", bufs=1) as vaugp,
            tc.tile_pool(name="rs", bufs=6) as rs,
            tc.tile_pool(name="pp", bufs=4) as pp,
            tc.tile_pool(name="rp", bufs=4) as rp,
            tc.tile_pool(name="bp", bufs=4) as bp,
            tc.tile_pool(name="yp", bufs=1) as yp,
            tc.tile_pool(name="op", bufs=3) as op,
            tc.tile_pool(name="ps1", bufs=4, space="PSUM") as ps1,
            tc.tile_pool(name="ps2", bufs=2, space="PSUM") as ps2,
        ):
            # ---- constants — interleaved with the first x tile so the
            # first qkv matmul can start after ~2 chunks
            wqk_sb = const.tile([128, KC, 512], BF16, tag="wqk")
            nc.sync.dma_start(wqk_sb[:], wqk[:])
            xt0 = xp.tile([128, KC, TQ], BF16, tag="xt", name="xt_0")
            nc.sync.dma_start(xt0[:], xT[:, 0, :, :])
            cos_sb = const.tile([128, T], F32, tag="cos")
            nc.sync.dma_start(cos_sb[:], cos4[:])
            sin_sb = const.tile([128, T], F32, tag="sin")
            nc.sync.dma_start(sin_sb[:], sing[:])
            wv_sb = const.tile([128, KC, 256], BF16, tag="wv")
            nc.sync.dma_start(wv_sb[:], wv[:])
            wp_sb = const.tile([128, 2, C], BF16, tag="wp")
            nc.sync.dma_start(wp_sb[:], wp[:])
            tri_sb = const.tile([128, 128], BF16, tag="tri")
            nc.sync.dma_start(tri_sb[:], trid[:])

            # persistent activations: q/k interleaved per head-pair group
            # (group g holds heads 2g, 2g+1; head hh of a group spans
            # partitions 64*hh .. 64*hh+64 with dims 0..63 in order)
            qg = [qkp.tile([128, T], BF16, tag=f"q{g}", name=f"q{g}")
                  for g in range(2)]
            kg = [qkp.tile([128, T], BF16, tag=f"k{g}", name=f"k{g}")
                  for g in range(2)]
            v_aug = vaugp.tile([128, 16, HPC * 65], BF16, tag="vaug")
            y0 = yp.tile([128, T], BF16, tag="y0")
            y1 = yp.tile([128, T], BF16, tag="y1")

            # ones columns of v_aug (col 64 of each head's 65-wide slot;
            # v occupies cols 0:64 so the AV result reads from the base-0
            # 64-partition window and sumexp lands on PSUM partition 64)
            for tb in range(16):
                va = v_aug[:, tb, :].rearrange("p (h c) -> p h c", c=65)
                nc.vector.memset(va[:, :, 64:65], 1.0)

            # prefetch remaining x tiles so qkv matmuls never wait on DMA
            xts = [xt0]
            for n in range(1, NT):
                tsl = bass.ts(n, TQ)
                xt = xp.tile([128, KC, TQ], BF16, tag="xt", name=f"xt_{n}")
                nc.sync.dma_start(xt[:], xT[:, n, :, :])
                xts.append(xt)

            # qkv projection + rope + v staging for token tile n
            def qkv_tile(n):
                tsl = bass.ts(n, TQ)
                xt = xts[n]

                # q groups pair then k groups pair, each in a 2-bank psum
                for (pair, dsts) in ((0, qg), (1, kg)):
                    pq = ps2.tile([128, 2 * TQ], F32, tag="ps2",
                                  name=f"qk_{n}_{pair}")
                    for half in range(2):
                        fb = 2 * pair + half
                        dst = pq[:, half * TQ:(half + 1) * TQ]
                        for kc in range(KC):
                            nc.tensor.matmul(
                                dst,
                                wqk_sb[:, kc, bass.ts(fb, 128)],
                                xt[:, kc, :],
                                start=(kc == 0),
                                stop=(kc == KC - 1),
                            )
                    # rope fused into psum eviction, emitting the
                    # interleaved layout:
                    #   out[r] = p[r]*cos[r] + p[r^32]*sing[r^32]
                    # (sing rows carry the rotate_half sign). The partition
                    # XOR-32 shuffle runs as four SBUF-to-SBUF DMAs per pair
                    # (both halves moved together), issued from the gpsimd /
                    # sync queues so DVE does only the two psum-evicting muls
                    # and one full-width 2x-rate bf16 add per block, and the
                    # qkv psum frees right after the muls.
                    cs = cos_sb[:, tsl]
                    sn = sin_sb[:, tsl]
                    t_cs = []
                    t_s = rs.tile([128, 2, TQ], BF16, tag="ts", name="t_s")
                    for half in range(2):
                        p_blk = pq[:, half * TQ:(half + 1) * TQ]
                        t_c = rs.tile([128, TQ], BF16, tag="t", name="t_c")
                        nc.vector.tensor_mul(t_c[:], p_blk, cs)
                        nc.vector.tensor_mul(t_s[:, half, :], p_blk, sn)
                        t_cs.append(t_c)
                    t_w = rs.tile([128, 2, TQ], BF16, tag="tw", name="t_w")
                    dma_q = nc.gpsimd if pair == 0 else nc.sync
                    for x0 in (0, 32, 64, 96):
                        xs = x0 ^ 32
                        dma_q.dma_start(t_w[x0:x0 + 32, :, :],
                                        t_s[xs:xs + 32, :, :])
                    for half in range(2):
                        nc.vector.tensor_add(dsts[half][:, tsl],
                                             t_cs[half][:], t_w[:, half, :])

                # v in [token, feat] layout: lhsT = xT chunk, rhs = w_v
                for tb in range(4):
                    blk = 4 * n + tb
                    pv = ps1.tile([128, 512], F32, tag="ps1",
                                  name=f"pv_{n}_{tb}")
                    for kc in range(KC):
                        nc.tensor.matmul(
                            pv[:, 0:256],
                            xt[:, kc, bass.ts(tb, 128)],
                            wv_sb[:, kc, :],
                            start=(kc == 0),
                            stop=(kc == KC - 1),
                        )
                    va = v_aug[:, blk, :].rearrange("p (h c) -> p h c", c=65)
                    pv_h = pv[:, 0:256].rearrange("p (h d) -> p h d", d=64)
                    nc.vector.tensor_copy(va[:, :, 0:64], pv_h[:])

            # attention + c_proj for query tile i (needs qkv tiles 0..i)
            def attn_tile(i, deferred):
                tq_sl = bass.ts(i, TQ)
                av_ps = [ps1.tile([65, TQ], F32, tag="ps1", name=f"av_{i}_{h}")
                         for h in range(HPC)]
                n_k = 4 * i + 4
                for kb in range(n_k):
                    ksl = bass.ts(kb, 128)
                    j = kb - 4 * i
                    last = kb == n_k - 1
                    skip = 128 * j if j > 0 else 0   # fully-masked lead cols
                    for pair in range(2):
                        s2 = ps2.tile([128, 2 * TQ], F32, tag="ps2",
                                      name=f"s_{i}_{kb}_{pair}")
                        for hh in range(2):
                            hsl = slice(64 * hh, 64 * hh + 64)
                            dst = s2[:, hh * TQ + skip:(hh + 1) * TQ]
                            qsl = bass.ds(i * TQ + skip, TQ - skip)
                            nc.tensor.matmul(dst, kg[pair][hsl, ksl],
                                             qg[pair][hsl, qsl],
                                             start=True, stop=True)
                        p_t = pp.tile([128, 2 * TQ], BF16, tag="p")
                        s2_v = s2[:].rearrange("p (g t) -> p g t", g=2)
                        pt_v = p_t[:].rearrange("p (g t) -> p g t", g=2)
                        nc.scalar.activation(pt_v[:, :, skip:TQ],
                                             s2_v[:, :, skip:TQ],
                                             Exp, scale=scale)
                        if j >= 0:
                            # 128-wide causal triangle: p *= (key <= tq)
                            for half in range(2):
                                off = half * TQ + skip
                                nc.vector.tensor_mul(
                                    p_t[:, off:off + 128],
                                    p_t[:, off:off + 128],
                                    tri_sb[:])
                        for hh in range(2):
                            h = 2 * pair + hh
                            nc.tensor.matmul(
                                av_ps[h][:, skip:],
                                v_aug[:, kb, bass.ts(h, 65)],
                                p_t[:, hh * TQ + skip:(hh + 1) * TQ],
                                start=(kb == 0),
                                stop=last,
                                skip_group_check=True,
                            )
                    if kb == 0:
                        for fn in deferred:
                            fn()
                        deferred.clear()

                # normalize: sumexp row to a base-0 SBUF tile (gpsimd copy,
                # off the DVE), 1/sumexp via fast reciprocal, then a gpsimd
                # partition-broadcast feeds the per-head scale mul
                for h in range(HPC):
                    se = rp.tile([1, TQ], F32, tag="se", name="se")
                    nc.scalar.copy(se[:], av_ps[h][64:65, :])
                    rec = rp.tile([1, TQ], F32, tag="r", name="rec")
                    nc.vector.reciprocal_approx_fast(rec[:], se[:])
                    bc_sb = bp.tile([64, TQ], F32, tag="bc")
                    nc.gpsimd.partition_broadcast(bc_sb[:], rec[:], channels=64)
                    y_t = y0 if h < 2 else y1
                    y_sl = y_t[(h % 2) * 64:(h % 2) * 64 + 64, tq_sl]
                    nc.vector.tensor_mul(y_sl, av_ps[h][0:64, :], bc_sb[:])

                # c_proj for the 4 finished token blocks; deferred so it
                # fills PE gaps during the NEXT tile instead of stalling
                def cproj():
                    for m in range(4 * i, 4 * i + 4):
                        msl = bass.ts(m, 128)
                        o_t = op.tile([128, C], BF16, tag="o")
                        for n2 in range(2):
                            nsl = bass.ts(n2, 512)
                            po = ps1.tile([128, 512], F32, tag="ps1",
                                          name=f"po_{m}_{n2}")
                            nc.tensor.matmul(po[:], y0[:, msl],
                                             wp_sb[:, 0, nsl],
                                             start=True, stop=False)
                            nc.tensor.matmul(po[:], y1[:, msl],
                                             wp_sb[:, 1, nsl],
                                             start=False, stop=True)
                            nc.vector.tensor_copy(o_t[:, nsl], po[:])
                        nc.sync.dma_start(out[msl, :], o_t[:])
                return cproj

            # emission order runs one qkv tile ahead of attention so the
            # rope chain of tile n+1 overlaps attention PE work of tile n
            qkv_tile(0)
            deferred = []
            for n in range(NT):
                if n + 1 < NT:
                    qkv_tile(n + 1)
                deferred = [attn_tile(n, deferred)]
            deferred[0]()

    nc.finalize()
    return nc


def _get_program():
    if "prog" not in _PROGRAM_CACHE:
        _PROGRAM_CACHE["prog"] = _build_program()
    return _PROGRAM_CACHE["prog"]


def _rope_tables_np():
    inv_freq = (1.0 / (10000.0 ** (np.arange(0, HD, 2, dtype=np.float32) / HD)))
    t = np.arange(T, dtype=np.float32)
    freqs = np.outer(t, inv_freq).astype(np.float32)      # [T, 32]
    cos4 = np.tile(np.cos(freqs).T, (4, 1))               # [128, T]
    sinT = np.sin(freqs).T                                # [32, T]
    sing = np.tile(np.vstack([sinT, -sinT]), (2, 1))      # [128, T] signed
    return np.ascontiguousarray(cos4), np.ascontiguousarray(sing)


def _core_inputs(x, W_attn, W_proj, bf16):
    """Per-core input dict list (shared-table entries reused, not copied)."""
    cos4, sing = _rope_tables_np()
    dd64 = np.arange(HD)
    tri = np.triu(np.ones((128, 128), dtype=np.float32)).astype(bf16)
    in_maps = []
    xTr = {}
    for b in range(B):
        # [p, n, kc, tq] with x.T[kc*128+p, n*TQ+tq]
        a = x[b].T.reshape(KC, 128, NT, TQ).transpose(1, 2, 0, 3)
        xTr[b] = np.ascontiguousarray(a.astype(bf16))
    for c in range(N_CORES):
        b = c // 4
        g = c % 4
        hs = 4 * g + np.arange(HPC)
        rows = (hs[:, None] * HD + dd64[None, :]).ravel()
        qk_cols = np.concatenate([rows, 1024 + rows])
        # [p, kc, f] with W[kc*128+p, col[f]]
        wqk = np.ascontiguousarray(
            W_attn[:, qk_cols].reshape(KC, 128, 512).astype(bf16))
        wv = np.ascontiguousarray(
            W_attn[:, 2048 + rows].reshape(KC, 128, 256).astype(bf16))
        wpr = np.ascontiguousarray(
            W_proj[rows, :].reshape(2, 128, C).transpose(1, 0, 2).astype(bf16))
        m = {
            "xT": xTr[b],
            "wqk": wqk.transpose(1, 0, 2).copy(),
            "wv": wv.transpose(1, 0, 2).copy(),
            "cos4": cos4,
            "sing": sing,
            "wp": wpr,
            "trid": tri,
        }
        in_maps.append(m)
    return in_maps


def _install_trace_shim():
    """Optional: lets run_bass_kernel_spmd(trace=True) capture NTFF profiles."""
    import contextlib
    import ctypes
    import types

    so = "/opt/axon/libaxon_pjrt.so"
    if not os.path.exists(so) or "antenv.axon_hooks" in sys.modules:
        return
    try:
        lib = ctypes.CDLL(so)
        if not hasattr(lib, "axon_start_nrt_profile"):
            return
        lib.axon_start_nrt_profile.argtypes = [ctypes.POINTER(ctypes.c_int64),
                                               ctypes.c_size_t]
        lib.axon_start_nrt_profile.restype = ctypes.c_int64
        lib.axon_stop_nrt_profile.argtypes = [ctypes.c_char_p]
        lib.axon_stop_nrt_profile.restype = ctypes.c_int64

        @contextlib.contextmanager
        def _hook(output_dir, device_ids):
            import jax
            jax.devices()
            if device_ids:
                ids = (ctypes.c_int64 * len(device_ids))(*device_ids)
                rc = lib.axon_start_nrt_profile(ids, len(device_ids))
            else:
                rc = lib.axon_start_nrt_profile(None, 0)
            if rc != 0:
                raise RuntimeError(f"axon_start_nrt_profile rc={rc}")
            try:
                yield
            finally:
                n = lib.axon_stop_nrt_profile(str(output_dir).encode())
                print(f"profile: {n} file(s) written to {output_dir}",
                      file=sys.stderr)

        mod = types.ModuleType("antenv.axon_hooks")
        mod.get_axon_ntff_profile_hook = lambda: _hook
        mod.set_axon_ntff_profile_hook = lambda h: None
        sys.modules["antenv.axon_hooks"] = mod
    except Exception:
        pass


def _kernel_numpy_fallback(x, W_attn, b_attn, W_proj, b_proj):
    """Reference math in numpy; only for nonzero b_attn (never hit by the
    harness, whose setup_inputs always passes zero biases)."""
    Bq, Tq, Cq = x.shape
    hd = Cq // H
    qkv = x @ W_attn + b_attn
    q, k, v = np.split(qkv, 3, axis=-1)
    to_heads = lambda a: a.reshape(Bq, Tq, H, hd).transpose(0, 2, 1, 3)
    q, k, v = to_heads(q), to_heads(k), to_heads(v)
    inv_freq = 1.0 / (10000.0 ** (np.arange(0, hd, 2, dtype=np.float32) / hd))
    t = np.arange(Tq, dtype=np.float32)
    freqs = np.outer(t, inv_freq)
    emb = np.concatenate([freqs, freqs], axis=-1)
    cos, sin = np.cos(emb), np.sin(emb)
    rot = lambda a: np.concatenate([-a[..., hd // 2:], a[..., :hd // 2]], -1)
    q = q * cos + rot(q) * sin
    k = k * cos + rot(k) * sin
    out = np.empty((Bq, H, Tq, hd), dtype=np.float32)
    causal = np.tril(np.ones((Tq, Tq), dtype=bool))
    for b in range(Bq):
        for h in range(H):
            s = (q[b, h] @ k[b, h].T) / np.sqrt(hd)
            s = np.where(causal, s, -np.inf)
            s -= s.max(-1, keepdims=True)
            p = np.exp(s)
            p /= p.sum(-1, keepdims=True)
            out[b, h] = p @ v[b, h]
    y = out.transpose(0, 2, 1, 3).reshape(Bq, Tq, Cq)
    return (y @ W_proj + b_proj).astype(np.float32)


def kernel(x, W_attn, b_attn, W_proj, b_proj):
    import ml_dtypes
    from concourse.bass_utils import run_bass_kernel_spmd

    bf16 = ml_dtypes.bfloat16

    x = np.asarray(x, dtype=np.float32)
    W_attn = np.asarray(W_attn, dtype=np.float32)
    b_attn = np.asarray(b_attn, dtype=np.float32)
    W_proj = np.asarray(W_proj, dtype=np.float32)
    b_proj = np.asarray(b_proj, dtype=np.float32)

    if np.any(b_attn):
        return _kernel_numpy_fallback(x, W_attn, b_attn, W_proj, b_proj)

    nc = _get_program()
    in_maps = _core_inputs(x, W_attn, W_proj, bf16)

    trace_dir = os.environ.get("BASSK_TRACE")
    kwargs = {}
    if trace_dir:
        _install_trace_shim()
        kwargs = {"trace": True, "tmpdir": trace_dir,
                  "trace_cores": [0], "stitch_traces": False}

    res = run_bass_kernel_spmd(nc, in_maps, core_ids=list(range(N_CORES)),
                               **kwargs)
    if trace_dir:
        kernel._last_result = res

    out = np.zeros((B, T, C), dtype=np.float32)
    for c in range(N_CORES):
        out[c // 4] += np.asarray(res.results[c]["out"], dtype=np.float32)
    if np.any(b_proj):
        out += b_proj
    return out
